# revision 5
# baseline (speedup 1.0000x reference)
"""CrossAttention (B=2, N=M=2048, 16 heads x 64) on 8 TRN2 NeuronCores. v2.

Sharding: data-parallel over batch (2) x tensor-parallel over heads (4 per
core). Each core computes q/k/v projections for its 4 heads, streaming
softmax(QK^T)V in a transposed (feature-major) layout, and a partial output
projection against its row-slice of Wo. Partial outputs are summed on host.

v2 schedule changes over v1 (sim 223.6us):
- column-split DMA rounds: first 512 query/key columns land ~6us earlier, so
  the prologue q/k chains and call A stream sooner.
- deadline-aware filler pacing: every projection chain carries the latest
  token-tile by which it must be emitted (so its consumer call never stalls),
  with uniform pacing otherwise.
- final(1) output block moved into call D (its deps complete at C's end),
  leaving D zero-idle; epilogue holds only pv(1,1)+final(2,3).
- epilogue: qkp pool closed after D frees 4 PSUM banks for a 4-deep final
  projection ring; output copies alternate ACT/DVE to double copy throughput;
  reciprocal_approx_fast replaces the iterative reciprocal.
- bf16 output partials (halves out-DMA; host accumulates in f32).
"""

import sys

if "/opt/trn_rl_repo" not in sys.path:
    sys.path.insert(0, "/opt/trn_rl_repo")

import ml_dtypes
import numpy as np

import concourse.bass as bass
import concourse.mybir as mybir
import concourse.tile as tile
from concourse import bacc
from concourse.bass_utils import run_bass_kernel_spmd

HEADS = 16
DH = 64
QD = 1024  # query/context feature dim
NN = 2048  # query tokens
MM = 2048  # context tokens
NCORES = 8
HPC = HEADS // (NCORES // 2)  # 4 heads per core
HD = HPC * DH  # 256 inner cols per core

BF = mybir.dt.bfloat16
F32 = mybir.dt.float32

_CACHE = {}

KT = QD // 128  # 8 contraction tiles for projections
TT = MM // 128  # 16 context-token tiles
IB = NN // 512  # 4 query-column blocks


def _build(repeat=1, out_bf=True):
    nc = bacc.Bacc("TRN2", target_bir_lowering=False, debug=False)
    xT = nc.declare_dram_parameter("xT", [QD, NN], BF, isOutput=False)
    cT = nc.declare_dram_parameter("cT", [QD, MM], BF, isOutput=False)
    wq = nc.declare_dram_parameter("wq", [QD, HD], BF, isOutput=False)
    wk = nc.declare_dram_parameter("wk", [QD, HD], BF, isOutput=False)
    wv = nc.declare_dram_parameter("wv", [QD, HD], BF, isOutput=False)
    wo = nc.declare_dram_parameter("wo", [HD, QD], BF, isOutput=False)
    out = nc.declare_dram_parameter("out", [QD, NN], BF if out_bf else F32,
                                    isOutput=True)
    with tile.TileContext(nc) as tc:
        for _ in range(repeat):
            _emit(tc, xT, cT, wq, wk, wv, wo, out)
    nc.compile()
    return nc


def _emit(tc, xT, cT, wq, wk, wv, wo, out):
    nc = tc.nc
    Exp = mybir.ActivationFunctionType.Exp
    mult = mybir.AluOpType.mult
    add = mybir.AluOpType.add
    I16 = mybir.dt.int16
    # bf16-domain Schraudolph exp for tail tiles: i16 = rne(x*A + B) bitcast
    # to bf16 approximates exp(0.125*x); constant multiplicative bias cancels
    # in softmax, residual ripple ~1.7% rms on ~3% of elements.
    SCH_A = float(np.float32(128.0 * 0.125) * np.float32(np.log2(np.e)))
    SCH_B = float(128.0 * (127.0 - 0.0434609))

    from contextlib import ExitStack
    ctx = ExitStack()
    persist = ctx.enter_context(tc.tile_pool(name="persist", bufs=1))
    xs = persist.tile([128, KT, NN], BF, tag="xs")
    cs = persist.tile([128, KT, MM], BF, tag="cs")
    wqs = persist.tile([128, KT, HD], BF, tag="wqs")
    wks = persist.tile([128, KT, HD], BF, tag="wks")
    wvs = persist.tile([128, KT, HD], BF, tag="wvs")
    wos = persist.tile([128, 2, QD], BF, tag="wos")
    qs = persist.tile([128, 2, NN], BF, tag="qs")  # qT: [head-pair, tokens]
    ks = persist.tile([128, 2, MM], BF, tag="ks")
    vs = persist.tile([128, TT, HPC, DH + 1], BF, tag="vs")  # v + ones col
    pvs = persist.tile([128, 2, NN], BF, tag="pvs")  # normalized attnV^T

    pvp = ctx.enter_context(tc.tile_pool(name="pv_ps", bufs=2, space="PSUM"))
    projp = ctx.enter_context(tc.tile_pool(name="proj_ps", bufs=2, space="PSUM"))
    qk_ctx = ExitStack()
    qkp = qk_ctx.enter_context(tc.tile_pool(name="qk_ps", bufs=2, space="PSUM"))
    expp = ctx.enter_context(tc.tile_pool(name="expp", bufs=37))
    outp = ctx.enter_context(tc.tile_pool(name="outp", bufs=6))
    nrm = ctx.enter_context(tc.tile_pool(name="nrm", bufs=2))

    # ---- loads: few fat DMAs (HWDGE costs ~625ns/instruction regardless of
    # size), ordered so call A's critical inputs land first at full HBM bw ----
    H2 = 512

    def ld(dst, srcT, c0, c1):
        nc.sync.dma_start(dst[:, :, c0:c1],
                          srcT[:, c0:c1].rearrange("(k p) c -> p k c", p=128))

    nc.sync.dma_start(wks[:, :, :], wk[:, :].rearrange("(k p) c -> p k c", p=128))
    ld(cs, cT, 0, H2)
    nc.sync.dma_start(wqs[:, :, :], wq[:, :].rearrange("(k p) c -> p k c", p=128))
    ld(xs, xT, 0, H2)
    ld(xs, xT, H2, 2 * H2)
    nc.sync.dma_start(wvs[:, :, :], wv[:, :].rearrange("(k p) c -> p k c", p=128))
    ld(cs, cT, H2, 2 * H2)
    ld(cs, cT, 1024, 2048)
    ld(xs, xT, 1024, 2048)
    nc.sync.dma_start(wos[:, :, :], wo[:, :].rearrange("(t p) c -> p t c", p=128))
    nc.gpsimd.memset(vs[:, :, :, DH:DH + 1], 1.0)

    def qk_chain(jb, i4, dst, w, src):
        ps = projp.tile([128, 512], F32, tag="proj", name="ps")
        for k in range(KT):
            nc.tensor.matmul(
                ps[:, :],
                lhsT=w[:, k, jb * 128:(jb + 1) * 128],
                rhs=src[:, k, i4 * 512:(i4 + 1) * 512],
                start=(k == 0),
                stop=(k == KT - 1),
            )
        nc.vector.tensor_copy(dst[:, jb, i4 * 512:(i4 + 1) * 512], ps[:, :])

    def v_chain(t2):
        # v projection for one token tile (token-major out: [tokens, hd])
        ps = projp.tile([128, HPC, DH], F32, tag="proj", name="ps")
        for k in range(KT):
            nc.tensor.matmul(
                ps[:, :, :],
                lhsT=cs[:, k, t2 * 128:(t2 + 1) * 128],
                rhs=wvs[:, k, :],
                start=(k == 0), stop=(k == KT - 1),
            )
        nc.vector.tensor_copy(vs[:, t2, :, 0:DH], ps[:, :, :])

    def attn(hp, ib2, fillers=(), dve_tail=0):
        """QK^T + exp for all 16 token tiles (2-head row-packed, K=64).
        fillers: list of (deadline_tt, cost_ns, fn); emitted when the deadline
        arrives, or earlier under uniform cost pacing."""
        rem = list(fillers)
        total = sum(c for _, _, c, _ in rem)
        spent = 0.0
        es = {}
        for t2 in range(TT):
            qk0 = qkp.tile([128, 1024], F32, tag="qk", name="qk0")
            qk1 = qkp.tile([128, 1024], F32, tag="qk", name="qk1")
            for i01 in range(2):
                c0 = ib2 * 1024 + i01 * 512
                nc.tensor.matmul(
                    qk0[:, i01 * 512:(i01 + 1) * 512],
                    lhsT=ks[0:64, hp, t2 * 128:(t2 + 1) * 128],
                    rhs=qs[0:64, hp, c0:c0 + 512],
                    start=True, stop=True,
                    tile_position=(0, 0),
                )
                nc.tensor.matmul(
                    qk1[:, i01 * 512:(i01 + 1) * 512],
                    lhsT=ks[64:128, hp, t2 * 128:(t2 + 1) * 128],
                    rhs=qs[64:128, hp, c0:c0 + 512],
                    start=True, stop=True,
                    tile_position=(64, 0),
                )
            e0 = expp.tile([128, 1024], BF, tag="exp", name="e0")
            e1 = expp.tile([128, 1024], BF, tag="exp", name="e1")
            if t2 >= TT - dve_tail:
                nc.vector.tensor_scalar(e0[:, :].bitcast(I16), qk0[:, :],
                                        SCH_A, SCH_B, mult, add)
                nc.vector.tensor_scalar(e1[:, :].bitcast(I16), qk1[:, :],
                                        SCH_A, SCH_B, mult, add)
            else:
                nc.scalar.activation(e0[:, :], qk0[:, :], Exp, scale=0.125)
                nc.scalar.activation(e1[:, :], qk1[:, :], Exp, scale=0.125)
            es[(t2, 0)], es[(t2, 1)] = e0, e1
            # deadline-forced first, then uniform pacing over ready items
            i = 0
            while i < len(rem):
                if rem[i][1] <= t2:
                    _, _, c, fn = rem.pop(i)
                    fn()
                    spent += c
                else:
                    i += 1
            target = total * (t2 + 1) / TT
            while spent < target:
                j = next((i for i, (r, _, _, _) in enumerate(rem) if r <= t2),
                         None)
                if j is None:
                    break
                _, _, c, fn = rem.pop(j)
                fn()
                spent += c
        while rem:
            _, _, c, fn = rem.pop(0)
            fn()
        return es

    def pv_units(hp, ib2, es, pool=None, rot=0):
        # PV + rowsum (M=65 augmented V) + normalize, as filler units.
        # Returns (h01=0 units, h01=1 units); callers splice other PE work
        # between the halves so the h1 chain's ring-WAR (waiting h0's norm
        # read) is hidden. rot rotates the accumulation order so the last
        # token-tile's matmul sits later in the chain than its exp.
        pool = pool or pvp
        halves = []
        order = [(t2 + rot) % TT for t2 in range(TT)]
        for h01 in range(2):
            cell = []

            def mm_unit(i, h01=h01, cell=cell):
                t2 = order[i]
                if i == 0:
                    cell.append([pool.tile([DH + 1, 512], F32, tag="pv",
                                           name="pv") for _ in range(2)])
                for i01 in range(2):
                    nc.tensor.matmul(
                        cell[0][i01][:, :],
                        lhsT=vs[:, t2, 2 * hp + h01, :],
                        rhs=es[(t2, h01)][:, i01 * 512:(i01 + 1) * 512],
                        start=(i == 0), stop=(i == TT - 1),
                    )

            def norm_unit(h01=h01, cell=cell):
                for i01 in range(2):
                    p = cell[0][i01]
                    c0 = ib2 * 1024 + i01 * 512
                    rc = nrm.tile([1, 512], F32, tag="rc", name="rc")
                    nc.vector.reciprocal(rc[:, :], p[64:65, :])
                    rep = nrm.tile([64, 512], F32, tag="rep", name="rep")
                    nc.gpsimd.partition_broadcast(rep[:, :], rc[:, :])
                    nc.vector.tensor_tensor(
                        pvs[h01 * 64:(h01 + 1) * 64, hp, c0:c0 + 512],
                        p[0:64, :],
                        rep[:, :],
                        mult,
                    )

            units = [(lambda i=i, f=mm_unit: f(i)) for i in range(TT)]
            units.append(norm_unit)
            halves.append(units)
        return halves

    def final_pair(ibp, ob, pool, engines=("dve", "dve")):
        # two adjacent 512-col output blocks for one row-block: 4 matmuls,
        # 2 copies, ONE fat DMA (HWDGE overhead is per-instruction)
        fps = [pool.tile([128, 512], F32, tag="proj", name=f"fp{i}")
               for i in range(2)]
        for i, ib in enumerate(ibp):
            for t2 in range(2):
                nc.tensor.matmul(
                    fps[i][:, :],
                    lhsT=wos[:, t2, ob * 128:(ob + 1) * 128],
                    rhs=pvs[:, t2, ib * 512:(ib + 1) * 512],
                    start=(t2 == 0), stop=(t2 == 1),
                )
        ot = outp.tile([128, 1024], out.dtype, tag="ot", name="ot")
        for i in range(2):
            if engines[i] == "act":
                nc.scalar.copy(ot[:, i * 512:(i + 1) * 512], fps[i][:, :])
            else:
                nc.vector.tensor_copy(ot[:, i * 512:(i + 1) * 512], fps[i][:, :])
        nc.sync.dma_start(
            out[ob * 128:(ob + 1) * 128, ibp[0] * 512:(ibp[0] + 2) * 512],
            ot[:, :])

    CH, VC, PV, FU = 1700, 850, 430, 430  # filler unit costs (ns)

    def chain_f(jb, i4, d, w, s):
        return lambda: qk_chain(jb, i4, d, w, s)

    # ---- prologue: the minimal chains call A reads immediately ----
    qk_chain(0, 0, ks, wks, cs)
    qk_chain(0, 0, qs, wqs, xs)
    qk_chain(0, 1, qs, wqs, xs)

    # ---- call A = (hp0, ib2=0) ----
    # k0(2)/k0(3) feed A's own tt>=8; k1/q1/q0(2,3) feed B/C; v feeds pv(0,0)
    fillA = [
        (0, 12, CH, chain_f(1, 0, ks, wks, cs)),
        (0, 14, CH, chain_f(1, 0, qs, wqs, xs)),
        (0, 15, CH, chain_f(1, 1, qs, wqs, xs)),
        (2, 3, CH, chain_f(0, 1, ks, wks, cs)),
    ] + [(2, 15, VC, (lambda t2=t2: v_chain(t2))) for t2 in range(4)] + [
        (3, 13, CH, chain_f(1, 1, ks, wks, cs)),
    ] + [(3, 15, VC, (lambda t2=t2: v_chain(t2))) for t2 in range(4, 8)] + [
        (6, 6, CH, chain_f(0, 2, ks, wks, cs)),
        (6, 10, CH, chain_f(0, 3, ks, wks, cs)),
    ] + [(6, 15, VC, (lambda t2=t2: v_chain(t2))) for t2 in range(8, TT)]
    esA = attn(0, 0, fillA)

    # ---- call B = (hp1, ib2=0): A's pv + remaining hp1 keys + C's queries --
    pvA = pv_units(0, 0, esA)
    fillB = [
        (0, 5, CH, chain_f(1, 2, ks, wks, cs)),
        (0, 9, CH, chain_f(1, 3, ks, wks, cs)),
    ] + [(0, 15, PV, u) for u in pvA[0]] + [
        (0, 11, CH, chain_f(0, 2, qs, wqs, xs)),
        (0, 13, CH, chain_f(0, 3, qs, wqs, xs)),
    ] + [(0, 15, PV, u) for u in pvA[1]]
    esB = attn(1, 0, fillB)

    # ---- call C = (hp0, ib2=1): B's pv + D's queries + final(0) ----
    pvB = pv_units(1, 0, esB)
    fillC = [
        (0, 10, CH, chain_f(1, 2, qs, wqs, xs)),
    ] + [(0, 15, PV, u) for u in pvB[0]] + [
        (0, 12, CH, chain_f(1, 3, qs, wqs, xs)),
    ] + [(0, 15, PV, u) for u in pvB[1]]
    esC = attn(0, 1, fillC)

    # ---- call D = (hp1, ib2=1): C's pv + final(1) ----
    pvC = pv_units(0, 1, esC)
    fillD = [(0, 15, PV, u) for u in pvC[0]] + [
        (0, 15, 2 * FU, (lambda ob=ob: final_pair((0, 1), ob, projp)))
        for ob in range(8)
    ] + [(0, 15, PV, u) for u in pvC[1]]
    esD = attn(1, 1, fillD)

    # ---- epilogue: D's pv, then final(2,3) on a 4-deep ring freed by qkp --
    qk_ctx.close()
    pvf = ctx.enter_context(tc.tile_pool(name="pvf", bufs=4, space="PSUM"))
    for half in pv_units(1, 1, esD, pool=pvf, rot=4):
        for u in half:
            u()
    for ob in range(8):
        final_pair((2, 3), ob, projp, engines=("dve", "act"))
    ctx.close()


def _inputs_for_core(c, x, context, Wq, Wk, Wv, Wo):
    bf = ml_dtypes.bfloat16
    b, g = c // (NCORES // 2), c % (NCORES // 2)
    sl = slice(g * HD, (g + 1) * HD)
    key = ("xc", b)
    if key not in _CACHE:
        _CACHE[key] = (
            np.ascontiguousarray(x[b].T).astype(bf),
            np.ascontiguousarray(context[b].T).astype(bf),
        )
    xTb, cTb = _CACHE[key]
    return {
        "xT": xTb,
        "cT": cTb,
        "wq": np.ascontiguousarray(Wq[:, sl]).astype(bf),
        "wk": np.ascontiguousarray(Wk[:, sl]).astype(bf),
        "wv": np.ascontiguousarray(Wv[:, sl]).astype(bf),
        "wo": np.ascontiguousarray(Wo[sl, :]).astype(bf),
    }


def kernel(x, context, Wq, Wk, Wv, Wo, bo):
    x = np.asarray(x, np.float32)
    context = np.asarray(context, np.float32)
    if "nc" not in _CACHE:
        _CACHE["nc"] = _build()
    _CACHE.pop(("xc", 0), None)
    _CACHE.pop(("xc", 1), None)
    nc = _CACHE["nc"]
    in_maps = [
        _inputs_for_core(c, x, context, np.asarray(Wq), np.asarray(Wk),
                         np.asarray(Wv), np.asarray(Wo))
        for c in range(NCORES)
    ]
    res = run_bass_kernel_spmd(nc, in_maps, list(range(NCORES))).results
    B = x.shape[0]
    G = NCORES // B
    outp = np.empty((B, NN, QD), np.float32)
    for b in range(B):
        acc = res[b * G]["out"].astype(np.float32)
        for g in range(1, G):
            acc = acc + res[b * G + g]["out"].astype(np.float32)
        outp[b] = acc.T + np.asarray(bo, np.float32)[None, :]
    return outp


# revision 6
# speedup vs baseline: 1.0129x; 1.0129x over previous
"""CrossAttention (B=2, N=M=2048, 16 heads x 64) on 8 TRN2 NeuronCores. v2.

Sharding: data-parallel over batch (2) x tensor-parallel over heads (4 per
core). Each core computes q/k/v projections for its 4 heads, streaming
softmax(QK^T)V in a transposed (feature-major) layout, and a partial output
projection against its row-slice of Wo. Partial outputs are summed on host.

v2 schedule changes over v1 (sim 223.6us):
- column-split DMA rounds: first 512 query/key columns land ~6us earlier, so
  the prologue q/k chains and call A stream sooner.
- deadline-aware filler pacing: every projection chain carries the latest
  token-tile by which it must be emitted (so its consumer call never stalls),
  with uniform pacing otherwise.
- final(1) output block moved into call D (its deps complete at C's end),
  leaving D zero-idle; epilogue holds only pv(1,1)+final(2,3).
- epilogue: qkp pool closed after D frees 4 PSUM banks for a 4-deep final
  projection ring; output copies alternate ACT/DVE to double copy throughput;
  reciprocal_approx_fast replaces the iterative reciprocal.
- bf16 output partials (halves out-DMA; host accumulates in f32).
"""

import sys

if "/opt/trn_rl_repo" not in sys.path:
    sys.path.insert(0, "/opt/trn_rl_repo")

import ml_dtypes
import numpy as np

import concourse.bass as bass
import concourse.mybir as mybir
import concourse.tile as tile
from concourse import bacc
from concourse.bass_utils import run_bass_kernel_spmd

HEADS = 16
DH = 64
QD = 1024  # query/context feature dim
NN = 2048  # query tokens
MM = 2048  # context tokens
NCORES = 8
HPC = HEADS // (NCORES // 2)  # 4 heads per core
HD = HPC * DH  # 256 inner cols per core

BF = mybir.dt.bfloat16
F32 = mybir.dt.float32

_CACHE = {}

KT = QD // 128  # 8 contraction tiles for projections
TT = MM // 128  # 16 context-token tiles
IB = NN // 512  # 4 query-column blocks


def _build(repeat=1, out_bf=True):
    nc = bacc.Bacc("TRN2", target_bir_lowering=False, debug=False)
    xT = nc.declare_dram_parameter("xT", [QD, NN], BF, isOutput=False)
    cT = nc.declare_dram_parameter("cT", [QD, MM], BF, isOutput=False)
    wq = nc.declare_dram_parameter("wq", [QD, HD], BF, isOutput=False)
    wk = nc.declare_dram_parameter("wk", [QD, HD], BF, isOutput=False)
    wv = nc.declare_dram_parameter("wv", [QD, HD], BF, isOutput=False)
    wo = nc.declare_dram_parameter("wo", [HD, QD], BF, isOutput=False)
    out = nc.declare_dram_parameter("out", [QD, NN], BF if out_bf else F32,
                                    isOutput=True)
    with tile.TileContext(nc) as tc:
        for _ in range(repeat):
            _emit(tc, xT, cT, wq, wk, wv, wo, out)
    nc.compile()
    return nc


def _emit(tc, xT, cT, wq, wk, wv, wo, out):
    nc = tc.nc
    Exp = mybir.ActivationFunctionType.Exp
    mult = mybir.AluOpType.mult
    add = mybir.AluOpType.add
    I16 = mybir.dt.int16
    # bf16-domain Schraudolph exp for tail tiles: i16 = rne(x*A + B) bitcast
    # to bf16 approximates exp(0.125*x); constant multiplicative bias cancels
    # in softmax, residual ripple ~1.7% rms on ~3% of elements.
    SCH_A = float(np.float32(128.0 * 0.125) * np.float32(np.log2(np.e)))
    SCH_B = float(128.0 * (127.0 - 0.0434609))

    from contextlib import ExitStack
    ctx = ExitStack()
    persist = ctx.enter_context(tc.tile_pool(name="persist", bufs=1))
    xs = persist.tile([128, KT, NN], BF, tag="xs")
    cs = persist.tile([128, KT, MM], BF, tag="cs")
    wqs = persist.tile([128, KT, HD], BF, tag="wqs")
    wks = persist.tile([128, KT, HD], BF, tag="wks")
    wvs = persist.tile([128, KT, HD], BF, tag="wvs")
    wos = persist.tile([128, 2, QD], BF, tag="wos")
    qs = persist.tile([128, 2, NN], BF, tag="qs")  # qT: [head-pair, tokens]
    ks = persist.tile([128, 2, MM], BF, tag="ks")
    vs = persist.tile([128, TT, HPC, DH + 1], BF, tag="vs")  # v + ones col
    pvs = persist.tile([128, 2, NN], BF, tag="pvs")  # normalized attnV^T

    pvp = ctx.enter_context(tc.tile_pool(name="pv_ps", bufs=2, space="PSUM"))
    projp = ctx.enter_context(tc.tile_pool(name="proj_ps", bufs=2, space="PSUM"))
    qk_ctx = ExitStack()
    qkp = qk_ctx.enter_context(tc.tile_pool(name="qk_ps", bufs=2, space="PSUM"))
    expp = ctx.enter_context(tc.tile_pool(name="expp", bufs=37))
    outp = ctx.enter_context(tc.tile_pool(name="outp", bufs=6))
    nrm = ctx.enter_context(tc.tile_pool(name="nrm", bufs=2))

    # ---- loads: few fat DMAs (HWDGE costs ~625ns/instruction regardless of
    # size), ordered so call A's critical inputs land first at full HBM bw ----
    H2 = 512

    def ld(dst, srcT, c0, c1):
        nc.sync.dma_start(dst[:, :, c0:c1],
                          srcT[:, c0:c1].rearrange("(k p) c -> p k c", p=128))

    nc.sync.dma_start(wks[:, :, :], wk[:, :].rearrange("(k p) c -> p k c", p=128))
    ld(cs, cT, 0, H2)
    nc.sync.dma_start(wqs[:, :, :], wq[:, :].rearrange("(k p) c -> p k c", p=128))
    ld(xs, xT, 0, H2)
    ld(xs, xT, H2, 2 * H2)
    nc.sync.dma_start(wvs[:, :, :], wv[:, :].rearrange("(k p) c -> p k c", p=128))
    ld(cs, cT, H2, 2 * H2)
    ld(cs, cT, 1024, 2048)
    ld(xs, xT, 1024, 2048)
    nc.sync.dma_start(wos[:, :, :], wo[:, :].rearrange("(t p) c -> p t c", p=128))
    nc.gpsimd.memset(vs[:, :, :, DH:DH + 1], 1.0)

    def qk_chain(jb, i4, dst, w, src):
        ps = projp.tile([128, 512], F32, tag="proj", name="ps")
        for k in range(KT):
            nc.tensor.matmul(
                ps[:, :],
                lhsT=w[:, k, jb * 128:(jb + 1) * 128],
                rhs=src[:, k, i4 * 512:(i4 + 1) * 512],
                start=(k == 0),
                stop=(k == KT - 1),
            )
        nc.vector.tensor_copy(dst[:, jb, i4 * 512:(i4 + 1) * 512], ps[:, :])

    def v_chain(t2):
        # v projection for one token tile (token-major out: [tokens, hd])
        ps = projp.tile([128, HPC, DH], F32, tag="proj", name="ps")
        for k in range(KT):
            nc.tensor.matmul(
                ps[:, :, :],
                lhsT=cs[:, k, t2 * 128:(t2 + 1) * 128],
                rhs=wvs[:, k, :],
                start=(k == 0), stop=(k == KT - 1),
            )
        nc.vector.tensor_copy(vs[:, t2, :, 0:DH], ps[:, :, :])

    def attn(hp, ib2, fillers=(), dve_tail=0):
        """QK^T + exp for all 16 token tiles (2-head row-packed, K=64).
        fillers: list of (deadline_tt, cost_ns, fn); emitted when the deadline
        arrives, or earlier under uniform cost pacing."""
        rem = list(fillers)
        total = sum(c for _, _, c, _ in rem)
        spent = 0.0
        es = {}
        for t2 in range(TT):
            qk0 = qkp.tile([128, 1024], F32, tag="qk", name="qk0")
            qk1 = qkp.tile([128, 1024], F32, tag="qk", name="qk1")
            for i01 in range(2):
                c0 = ib2 * 1024 + i01 * 512
                nc.tensor.matmul(
                    qk0[:, i01 * 512:(i01 + 1) * 512],
                    lhsT=ks[0:64, hp, t2 * 128:(t2 + 1) * 128],
                    rhs=qs[0:64, hp, c0:c0 + 512],
                    start=True, stop=True,
                    tile_position=(0, 0),
                )
                nc.tensor.matmul(
                    qk1[:, i01 * 512:(i01 + 1) * 512],
                    lhsT=ks[64:128, hp, t2 * 128:(t2 + 1) * 128],
                    rhs=qs[64:128, hp, c0:c0 + 512],
                    start=True, stop=True,
                    tile_position=(64, 0),
                )
            e0 = expp.tile([128, 1024], BF, tag="exp", name="e0")
            e1 = expp.tile([128, 1024], BF, tag="exp", name="e1")
            if t2 >= TT - dve_tail:
                nc.vector.tensor_scalar(e0[:, :].bitcast(I16), qk0[:, :],
                                        SCH_A, SCH_B, mult, add)
                nc.vector.tensor_scalar(e1[:, :].bitcast(I16), qk1[:, :],
                                        SCH_A, SCH_B, mult, add)
            else:
                nc.scalar.activation(e0[:, :], qk0[:, :], Exp, scale=0.125)
                nc.scalar.activation(e1[:, :], qk1[:, :], Exp, scale=0.125)
            es[(t2, 0)], es[(t2, 1)] = e0, e1
            # deadline-forced first, then uniform pacing over ready items
            i = 0
            while i < len(rem):
                if rem[i][1] <= t2:
                    _, _, c, fn = rem.pop(i)
                    fn()
                    spent += c
                else:
                    i += 1
            target = total * (t2 + 1) / TT
            while spent < target:
                j = next((i for i, (r, _, _, _) in enumerate(rem) if r <= t2),
                         None)
                if j is None:
                    break
                _, _, c, fn = rem.pop(j)
                fn()
                spent += c
        while rem:
            _, _, c, fn = rem.pop(0)
            fn()
        return es

    def pv_units(hp, ib2, es, pool=None, rot=0):
        # PV + rowsum (M=65 augmented V) + normalize, as filler units.
        # Returns (h01=0 units, h01=1 units); callers splice other PE work
        # between the halves so the h1 chain's ring-WAR (waiting h0's norm
        # read) is hidden. rot rotates the accumulation order so the last
        # token-tile's matmul sits later in the chain than its exp.
        pool = pool or pvp
        halves = []
        order = [(t2 + rot) % TT for t2 in range(TT)]
        for h01 in range(2):
            cell = []

            def mm_unit(i, h01=h01, cell=cell):
                t2 = order[i]
                if i == 0:
                    cell.append([pool.tile([DH + 1, 512], F32, tag="pv",
                                           name="pv") for _ in range(2)])
                for i01 in range(2):
                    nc.tensor.matmul(
                        cell[0][i01][:, :],
                        lhsT=vs[:, t2, 2 * hp + h01, :],
                        rhs=es[(t2, h01)][:, i01 * 512:(i01 + 1) * 512],
                        start=(i == 0), stop=(i == TT - 1),
                    )

            def norm_unit(h01=h01, cell=cell):
                for i01 in range(2):
                    p = cell[0][i01]
                    c0 = ib2 * 1024 + i01 * 512
                    rc = nrm.tile([1, 512], F32, tag="rc", name="rc")
                    nc.vector.reciprocal(rc[:, :], p[64:65, :])
                    rep = nrm.tile([64, 512], F32, tag="rep", name="rep")
                    nc.gpsimd.partition_broadcast(rep[:, :], rc[:, :])
                    nc.vector.tensor_tensor(
                        pvs[h01 * 64:(h01 + 1) * 64, hp, c0:c0 + 512],
                        p[0:64, :],
                        rep[:, :],
                        mult,
                    )

            units = [(lambda i=i, f=mm_unit: f(i)) for i in range(TT)]
            units.append(norm_unit)
            halves.append(units)
        return halves

    def final_pair(ibp, ob, pool, engines=("dve", "dve")):
        # two adjacent 512-col output blocks for one row-block: 4 matmuls,
        # 2 copies, ONE fat DMA (HWDGE overhead is per-instruction)
        fps = [pool.tile([128, 512], F32, tag="proj", name=f"fp{i}")
               for i in range(2)]
        for i, ib in enumerate(ibp):
            for t2 in range(2):
                nc.tensor.matmul(
                    fps[i][:, :],
                    lhsT=wos[:, t2, ob * 128:(ob + 1) * 128],
                    rhs=pvs[:, t2, ib * 512:(ib + 1) * 512],
                    start=(t2 == 0), stop=(t2 == 1),
                )
        ot = outp.tile([128, 1024], out.dtype, tag="ot", name="ot")
        for i in range(2):
            if engines[i] == "act":
                nc.scalar.copy(ot[:, i * 512:(i + 1) * 512], fps[i][:, :])
            else:
                nc.vector.tensor_copy(ot[:, i * 512:(i + 1) * 512], fps[i][:, :])
        nc.sync.dma_start(
            out[ob * 128:(ob + 1) * 128, ibp[0] * 512:(ibp[0] + 2) * 512],
            ot[:, :])

    CH, VC, PV, FU = 1700, 850, 430, 430  # filler unit costs (ns)

    def chain_f(jb, i4, d, w, s):
        return lambda: qk_chain(jb, i4, d, w, s)

    # ---- prologue: the minimal chains call A reads immediately ----
    qk_chain(0, 0, ks, wks, cs)
    qk_chain(0, 0, qs, wqs, xs)
    qk_chain(0, 1, qs, wqs, xs)

    # ---- call A = (hp0, ib2=0) ----
    # k0(2)/k0(3) feed A's own tt>=8; k1/q1/q0(2,3) feed B/C; v feeds pv(0,0)
    fillA = [
        (0, 12, CH, chain_f(1, 0, ks, wks, cs)),
        (0, 14, CH, chain_f(1, 0, qs, wqs, xs)),
        (0, 15, CH, chain_f(1, 1, qs, wqs, xs)),
        (2, 3, CH, chain_f(0, 1, ks, wks, cs)),
    ] + [(2, 15, VC, (lambda t2=t2: v_chain(t2))) for t2 in range(4)] + [
        (3, 13, CH, chain_f(1, 1, ks, wks, cs)),
    ] + [(3, 15, VC, (lambda t2=t2: v_chain(t2))) for t2 in range(4, 8)] + [
        (6, 6, CH, chain_f(0, 2, ks, wks, cs)),
        (6, 10, CH, chain_f(0, 3, ks, wks, cs)),
    ] + [(6, 15, VC, (lambda t2=t2: v_chain(t2))) for t2 in range(8, TT)]
    esA = attn(0, 0, fillA)

    # ---- call B = (hp1, ib2=0): A's pv + remaining hp1 keys + C's queries --
    pvA = pv_units(0, 0, esA)
    fillB = [
        (0, 5, CH, chain_f(1, 2, ks, wks, cs)),
        (0, 9, CH, chain_f(1, 3, ks, wks, cs)),
    ] + [(0, 15, PV, u) for u in pvA[0]] + [
        (0, 11, CH, chain_f(0, 2, qs, wqs, xs)),
        (0, 13, CH, chain_f(0, 3, qs, wqs, xs)),
    ] + [(0, 15, PV, u) for u in pvA[1]]
    esB = attn(1, 0, fillB)

    # ---- call C = (hp0, ib2=1): B's pv + D's queries + final(0) ----
    pvB = pv_units(1, 0, esB)
    fillC = [
        (0, 10, CH, chain_f(1, 2, qs, wqs, xs)),
    ] + [(0, 15, PV, u) for u in pvB[0]] + [
        (0, 12, CH, chain_f(1, 3, qs, wqs, xs)),
    ] + [(0, 15, PV, u) for u in pvB[1]]
    esC = attn(0, 1, fillC)

    # ---- call D = (hp1, ib2=1): C's pv + final(1) ----
    pvC = pv_units(0, 1, esC)
    fillD = [(0, 15, PV, u) for u in pvC[0]] + [
        (0, 15, 2 * FU, (lambda ob=ob: final_pair((0, 1), ob, projp)))
        for ob in range(8)
    ] + [(0, 15, PV, u) for u in pvC[1]]
    esD = attn(1, 1, fillD)

    # ---- epilogue: D's pv, then final(2,3) on a 4-deep ring freed by qkp --
    qk_ctx.close()
    projf = ctx.enter_context(tc.tile_pool(name="projf", bufs=4, space="PSUM"))
    for half in pv_units(1, 1, esD, rot=4):
        for u in half:
            u()
    for ob in range(8):
        final_pair((2, 3), ob, projf, engines=("dve", "act"))
    ctx.close()


def _inputs_for_core(c, x, context, Wq, Wk, Wv, Wo):
    bf = ml_dtypes.bfloat16
    b, g = c // (NCORES // 2), c % (NCORES // 2)
    sl = slice(g * HD, (g + 1) * HD)
    key = ("xc", b)
    if key not in _CACHE:
        _CACHE[key] = (
            np.ascontiguousarray(x[b].T).astype(bf),
            np.ascontiguousarray(context[b].T).astype(bf),
        )
    xTb, cTb = _CACHE[key]
    return {
        "xT": xTb,
        "cT": cTb,
        "wq": np.ascontiguousarray(Wq[:, sl]).astype(bf),
        "wk": np.ascontiguousarray(Wk[:, sl]).astype(bf),
        "wv": np.ascontiguousarray(Wv[:, sl]).astype(bf),
        "wo": np.ascontiguousarray(Wo[sl, :]).astype(bf),
    }


def kernel(x, context, Wq, Wk, Wv, Wo, bo):
    x = np.asarray(x, np.float32)
    context = np.asarray(context, np.float32)
    if "nc" not in _CACHE:
        _CACHE["nc"] = _build()
    _CACHE.pop(("xc", 0), None)
    _CACHE.pop(("xc", 1), None)
    nc = _CACHE["nc"]
    in_maps = [
        _inputs_for_core(c, x, context, np.asarray(Wq), np.asarray(Wk),
                         np.asarray(Wv), np.asarray(Wo))
        for c in range(NCORES)
    ]
    res = run_bass_kernel_spmd(nc, in_maps, list(range(NCORES))).results
    B = x.shape[0]
    G = NCORES // B
    outp = np.empty((B, NN, QD), np.float32)
    for b in range(B):
        acc = res[b * G]["out"].astype(np.float32)
        for g in range(1, G):
            acc = acc + res[b * G + g]["out"].astype(np.float32)
        outp[b] = acc.T + np.asarray(bo, np.float32)[None, :]
    return outp
